# revision 36
# baseline (speedup 1.0000x reference)
"""PraxisSMEAR (soft-merged MoE GLU MLP) on 8 TRN2 NeuronCores.

Strategy: data-parallel over tokens (1024 per core).  The expert weight
merge (weighted sum over E=8 experts) is sharded over the contraction
dims (d for W_up, f for W_dn).  Merged bf16 weights are AllGathered in
8 o-chunks so the up GEMM starts as soon as the first chunk lands; the
first 6 chunks merge on the TensorEngine (fast path for the first
AllGather), the rest on Scalar+Vector engines so they overlap the GEMM.
Router runs on-device with a tiny [4,8] AllReduce of partial logits.
Main GEMMs run in bf16 with f32 PSUM accumulation.

Host-side work is limited to sharding/relayout: transposes, an
o-permutation of W_up rows (so linear/gated GLU tiles are adjacent),
f32->bf16 casts, and the final transpose-concat of per-core outputs.
"""

import numpy as np
import ml_dtypes

from concourse import bacc, tile
import concourse.bass as bass
import concourse.mybir as mybir
from concourse.bass import ds, ts
from concourse.bass_utils import run_bass_kernel_spmd
from concourse.masks import make_identity

NCORES = 8
P = 128
B, S, D = 4, 2048, 1024
T = B * S               # 8192 tokens total
TPC = T // NCORES       # 1024 tokens per core
E = 8
DFF = 8 * D             # 8192
F = 4 * D               # 4096 (GLU half)
KS = D // P             # 8 k-subtiles of 128 over d
FKS = F // P            # 32 k-subtiles of 128 over f
RWB_ROWS = D + P        # router_w.T rows padded: row D = router_b
# AllGather chunks over the o' axis: (first 512-step, number of 512-steps).
# Small first chunks get the up GEMM started as early as possible.
AG_CHUNKS = [(0, 4), (4, 5), (9, 7)]
N_PE_MERGE_STEPS = 8    # up-merge 512-steps done on PE (rest on DVE/ACT)

F32 = mybir.dt.float32
BF16 = mybir.dt.bfloat16

_CACHED = {}


def _build():
    nc = bacc.Bacc("TRN2", target_bir_lowering=False, debug=False,
                   num_devices=NCORES)

    xt = nc.dram_tensor("xt", [D, TPC], F32, kind="ExternalInput")
    rwb = nc.dram_tensor("rwb", [RWB_ROWS, E], F32, kind="ExternalInput")
    upw = nc.dram_tensor("upw", [DFF // 512, P, E, 512], BF16,
                         kind="ExternalInput")
    dnw = nc.dram_tensor("dnw", [4, 2, P, E, 512], BF16, kind="ExternalInput")
    upb = nc.dram_tensor("upb", [P, DFF], F32, kind="ExternalInput")
    dnb = nc.dram_tensor("dnb", [P, D], F32, kind="ExternalInput")
    bsel = nc.dram_tensor("bsel", [P, B], F32, kind="ExternalInput")
    out = nc.dram_tensor("out", [D, TPC], F32, kind="ExternalOutput")

    rg = [list(range(NCORES))]

    with tile.TileContext(nc) as tc:
        with (
            tc.tile_pool(name="const", bufs=1) as const,
            tc.tile_pool(name="dram", bufs=1, space="DRAM") as dram,
        ):
            # ---- internal DRAM (collective bounce buffers) ----
            ar_in = dram.tile([B, E], F32)
            ar_out = dram.tile([B, E], F32, addr_space="Shared")
            agu_in = [dram.tile([n, P, 512], BF16, name=f"agu_in{i}")
                      for i, (_, n) in enumerate(AG_CHUNKS)]
            agu_out = [dram.tile([NCORES, n, P, 512], BF16,
                                 addr_space="Shared", name=f"agu_out{i}")
                       for i, (_, n) in enumerate(AG_CHUNKS)]
            agd_in = dram.tile([4, 2, P, 512], BF16)
            agd_out = dram.tile([NCORES, 4, 2, P, 512], BF16,
                                addr_space="Shared")
            bup_dram = dram.tile([DFF], F32)
            bdn_dram = dram.tile([D], F32)

            # ---- persistent SBUF ----
            xt_bf = const.tile([P, KS, TPC], BF16)
            id_bf = const.tile([P, P], BF16)
            make_identity(nc, id_bf)
            id_f32 = const.tile([P, P], F32)
            make_identity(nc, id_f32)
            id_scaled = const.tile([P, E, P], BF16)
            probs_bcast = const.tile([P, E], F32)
            probs_col = const.tile([P, 1], F32)
            bup_sb = const.tile([P, DFF // P], F32)
            bdn_sb = const.tile([P, D // P], F32)
            bsel_sb = const.tile([P, B], F32)
            e0_ones = const.tile([P, P], F32)
            nc.vector.memset(e0_ones[:], 0.0)
            nc.vector.memset(e0_ones[0:1, :], 1.0)
            quarter_all = const.tile([P, P], F32)
            nc.vector.memset(quarter_all[:], 0.0)
            nc.vector.memset(quarter_all[0:B, :], 1.0 / B)

            nc.sync.dma_start(bsel_sb[:], bsel[:])

            # ================= router =================
            with (
                tc.tile_pool(name="rstage", bufs=3) as rstage,
                tc.tile_pool(name="rmisc", bufs=1) as rmisc,
                tc.tile_pool(name="rpsum", bufs=1, space="PSUM") as rpsum,
            ):
                red4 = rmisc.tile([P, KS, 4], F32)
                NCH = TPC // 4
                for j in range(4):
                    xs = rstage.tile([P, KS, NCH], F32)
                    nc.sync.dma_start(
                        xs[:],
                        xt.rearrange("(ks p) t -> p ks t", p=P)[:, :, ds(j * NCH, NCH)],
                    )
                    nc.vector.reduce_sum(red4[:, :, j:j + 1], xs[:],
                                         axis=mybir.AxisListType.X)
                    # cast off the router critical path (ScalarE)
                    nc.scalar.activation(xt_bf[:, :, ds(j * NCH, NCH)], xs[:],
                                         mybir.ActivationFunctionType.Copy,
                                         bias=0.0, scale=1.0)

                red = rmisc.tile([P, KS + 1, 1], F32)
                nc.vector.reduce_sum(red[:, :KS, :], red4[:],
                                     axis=mybir.AxisListType.X)
                nc.vector.tensor_scalar_mul(red[:, :KS, :], red[:, :KS, :], 1.0 / S)
                # bias row: after the 8-core AllReduce each batch gets 2
                # contributions, so 0.5 * router_b * 2 = router_b
                nc.vector.memset(red[:, KS, :], 0.0)
                nc.vector.memset(red[0:1, KS, :], 0.5)

                sel = rmisc.tile([P, KS + 1, B], F32)
                for kt in range(KS + 1):
                    nc.vector.tensor_tensor(
                        sel[:, kt, :],
                        red[:, kt, :].to_broadcast([P, B]),
                        bsel_sb[:],
                        mybir.AluOpType.mult,
                    )

                rwb_sb = rmisc.tile([P, KS + 1, E], F32)
                nc.sync.dma_start(rwb_sb[:], rwb.rearrange("(ks p) e -> p ks e", p=P))

                ps_l = rpsum.tile([B, E], F32)
                for kt in range(KS + 1):
                    nc.tensor.matmul(ps_l[:], sel[:, kt, :], rwb_sb[:, kt, :],
                                     start=(kt == 0), stop=(kt == KS))
                logits_part = rmisc.tile([B, E], F32)
                nc.vector.tensor_copy(logits_part[:], ps_l[:])
                nc.scalar.dma_start(ar_in[:], logits_part[:])
                nc.gpsimd.collective_compute(
                    "AllReduce", mybir.AluOpType.add, replica_groups=rg,
                    ins=[ar_in[:]], outs=[ar_out[:]],
                )
                logits = rmisc.tile([B, E], F32)
                nc.scalar.dma_start(logits[:], ar_out[:])

                # softmax over E per batch, then mean over batches.
                # logits are O(1) here (x ~ N(0,1) means), so no max-
                # subtraction is needed for a stable exp in f32.
                probs_pad = rmisc.tile([P, E], F32)
                nc.vector.memset(probs_pad[:], 0.0)
                nc.scalar.activation(probs_pad[0:B, :], logits[:],
                                     mybir.ActivationFunctionType.Exp,
                                     bias=0.0, scale=1.0)
                sm = rmisc.tile([B, 1], F32)
                nc.vector.reduce_sum(sm[:], probs_pad[0:B, :],
                                     axis=mybir.AxisListType.X)
                rc = rmisc.tile([B, 1], F32)
                nc.vector.reciprocal(rc[:], sm[:])
                nc.vector.tensor_scalar_mul(probs_pad[0:B, :], probs_pad[0:B, :],
                                            rc[:])

                # fused batch-mean + broadcast to all 128 partitions:
                # quarter_all rows 0..3 are 1/B, so lhsT.T @ probs_pad
                # yields the batch mean replicated on every partition
                ps_pb = rpsum.tile([P, E], F32)
                nc.tensor.matmul(ps_pb[:], quarter_all[:], probs_pad[:],
                                 start=True, stop=True)
                nc.vector.tensor_copy(probs_bcast[:], ps_pb[:])

                # scaled identities for the merge matmuls
                for e in range(E):
                    nc.vector.tensor_scalar_mul(id_scaled[:, e, :], id_bf[:],
                                                probs_bcast[:, e:e + 1])
                # probs as a column vector (rows 0..7), for bias merges
                nc.vector.memset(probs_col[:], 0.0)
                tmp8 = rmisc.tile([E, E], F32)
                nc.vector.tensor_tensor(tmp8[:], probs_bcast[0:E, :],
                                        id_f32[0:E, 0:E], mybir.AluOpType.mult)
                nc.vector.reduce_sum(probs_col[0:E, :], tmp8[:],
                                     axis=mybir.AxisListType.X)

            # shared pools for merge helpers (open until end of GEMMs)
            from contextlib import ExitStack
            mstack = ExitStack()
            dvem = mstack.enter_context(tc.tile_pool(name="dvem", bufs=4))
            dvacc = mstack.enter_context(tc.tile_pool(name="dvacc", bufs=2))
            mev = mstack.enter_context(tc.tile_pool(name="mev", bufs=6))

            def merge_step_pe(pool, mpsum, src, dst):
                win = pool.tile([P, E, 512], BF16, tag="win")
                nc.sync.dma_start(win[:], src)
                ps = mpsum.tile([P, 512], F32, tag="mps")
                for e in range(E):
                    nc.tensor.matmul(ps[:], id_scaled[:, e, :], win[:, e, :],
                                     start=(e == 0), stop=(e == E - 1))
                ev = mev.tile([P, 512], BF16, tag="mev")
                nc.vector.tensor_copy(ev[:], ps[:])
                nc.gpsimd.dma_start(dst, ev[:])

            def merge_step_dve(pool, src, dst):
                win = pool.tile([P, E, 512], BF16, tag="win")
                nc.scalar.dma_start(win[:], src)
                acc = dvacc.tile([P, 512], F32, tag="acc")
                nc.scalar.activation(acc[:], win[:, 0, :],
                                     mybir.ActivationFunctionType.Copy,
                                     bias=0.0, scale=probs_bcast[:, 0:1])
                for e in range(1, E):
                    tmp = dvem.tile([P, 512], F32, tag="mtmp")
                    nc.scalar.activation(tmp[:], win[:, e, :],
                                         mybir.ActivationFunctionType.Copy,
                                         bias=0.0, scale=probs_bcast[:, e:e + 1])
                    nc.vector.tensor_tensor(acc[:], acc[:], tmp[:],
                                            mybir.AluOpType.add)
                ev = mev.tile([P, 512], BF16, tag="mev")
                nc.vector.tensor_copy(ev[:], acc[:])
                nc.gpsimd.dma_start(dst, ev[:])

            # ============ up-merge + chunked AllGather ============
            with (
                tc.tile_pool(name="upwin", bufs=10) as upwin,
                tc.tile_pool(name="mpsum", bufs=3, space="PSUM") as mpsum,
            ):
                # all up chunks merge on PE, before the GEMM
                for ci, (s0, nsteps) in enumerate(AG_CHUNKS):
                    for k in range(nsteps):
                        j = s0 + k
                        merge_step_pe(upwin, mpsum, upw[j], agu_in[ci][k])
                    nc.gpsimd.collective_compute(
                        "AllGather", mybir.AluOpType.bypass,
                        replica_groups=rg,
                        ins=[agu_in[ci][:]], outs=[agu_out[ci][:]],
                    )

                # ============ bias merges (PE, tiny) ============
                with (
                    tc.tile_pool(name="bias", bufs=2) as biasp,
                    tc.tile_pool(name="bpsum", bufs=2, space="PSUM") as bpsum,
                ):
                    for j in range(DFF // 512):
                        ub = biasp.tile([P, 512], F32, tag="ub")
                        nc.sync.dma_start(ub[:], upb[:, ds(j * 512, 512)])
                        pb = bpsum.tile([1, 512], F32, tag="pb")
                        nc.tensor.matmul(pb[:], probs_col[:], ub[:],
                                         start=True, stop=True)
                        bev = biasp.tile([1, 512], F32, tag="bev")
                        nc.vector.tensor_copy(bev[:], pb[:])
                        nc.gpsimd.dma_start(bup_dram[None, ds(j * 512, 512)],
                                            bev[:])
                    for j in range(D // 512):
                        db = biasp.tile([P, 512], F32, tag="ub")
                        nc.sync.dma_start(db[:], dnb[:, ds(j * 512, 512)])
                        pb = bpsum.tile([1, 512], F32, tag="pb")
                        nc.tensor.matmul(pb[:], probs_col[:], db[:],
                                         start=True, stop=True)
                        bev = biasp.tile([1, 512], F32, tag="bev")
                        nc.vector.tensor_copy(bev[:], pb[:])
                        nc.gpsimd.dma_start(bdn_dram[None, ds(j * 512, 512)],
                                            bev[:])

                    nc.scalar.dma_start(bup_sb[:],
                                      bup_dram.rearrange("(ot p) -> p ot", p=P))
                    nc.scalar.dma_start(bdn_sb[:],
                                      bdn_dram.rearrange("(dt p) -> p dt", p=P))

            # ================= GEMMs =================
            # o' layout (host-permuted): o'-tile 2t = linear f-tile t,
            # o'-tile 2t+1 = gated f-tile t.
            with (
                tc.tile_pool(name="stpool", bufs=1) as stpool,
                tc.tile_pool(name="dnwin", bufs=4) as dnwin,
            ):
                st_a = stpool.tile([P, FKS // 2, TPC], BF16)
                st_b = stpool.tile([P, FKS // 2, TPC], BF16)

                def st_slice(ft, sl):
                    if ft < FKS // 2:
                        return st_a[:, ft, sl]
                    return st_b[:, ft - FKS // 2, sl]

                def up_gemm_chunk(wstrip, glu, gpsum, ci, after_pair=None):
                    s0, nsteps = AG_CHUNKS[ci]
                    for h in range(nsteps):
                        oc = s0 + h
                        wsu = wstrip.tile([P, KS, 512], BF16, tag="wsu")
                        for kt in range(KS):
                            nc.sync.dma_start(wsu[:, kt, :], agu_out[ci][kt, h])
                        for j2 in range(2):
                            ft = 2 * oc + j2
                            lhs_l = wsu[:, :, ds(j2 * 256, P)]
                            lhs_g = wsu[:, :, ds(j2 * 256 + P, P)]
                            ps_l0 = gpsum.tile([P, 512], F32, tag="ps_l0")
                            ps_l1 = gpsum.tile([P, 512], F32, tag="ps_l1")
                            ps_g0 = gpsum.tile([P, 512], F32, tag="ps_g0")
                            ps_g1 = gpsum.tile([P, 512], F32, tag="ps_g1")
                            for kt in range(KS):
                                nc.tensor.matmul(ps_l0[:], lhs_l[:, kt, :],
                                                 xt_bf[:, kt, 0:512],
                                                 start=(kt == 0),
                                                 stop=(kt == KS - 1))
                                nc.tensor.matmul(ps_l1[:], lhs_l[:, kt, :],
                                                 xt_bf[:, kt, 512:1024],
                                                 start=(kt == 0),
                                                 stop=(kt == KS - 1))
                            for kt in range(KS):
                                nc.tensor.matmul(ps_g0[:], lhs_g[:, kt, :],
                                                 xt_bf[:, kt, 0:512],
                                                 start=(kt == 0),
                                                 stop=(kt == KS - 1))
                                nc.tensor.matmul(ps_g1[:], lhs_g[:, kt, :],
                                                 xt_bf[:, kt, 512:1024],
                                                 start=(kt == 0),
                                                 stop=(kt == KS - 1))
                            for th, ps_lx, ps_gx in ((0, ps_l0, ps_g0),
                                                     (1, ps_l1, ps_g1)):
                                lin = glu.tile([P, 512], F32, tag="lin")
                                nc.vector.tensor_scalar_add(
                                    lin[:], ps_lx[:],
                                    bup_sb[:, 2 * ft:2 * ft + 1])
                                sil = glu.tile([P, 512], F32, tag="sil")
                                nc.scalar.activation(
                                    sil[:], ps_gx[:],
                                    mybir.ActivationFunctionType.Silu,
                                    bias=bup_sb[:, 2 * ft + 1:2 * ft + 2],
                                    scale=1.0)
                                nc.vector.tensor_tensor(
                                    st_slice(ft, ds(th * 512, 512)),
                                    lin[:], sil[:], mybir.AluOpType.mult)
                            if after_pair is not None:
                                after_pair()

                with (
                    tc.tile_pool(name="wstrip", bufs=3) as wstrip,
                    tc.tile_pool(name="glu", bufs=3) as glu,
                    tc.tile_pool(name="gpsum", bufs=2, space="PSUM") as gpsum,
                ):
                    # down-merge steps (DVE/ACT), spread one per GEMM
                    # pair-group so they never starve the GLU evictions
                    dn_steps = [(fq, h) for fq in range(4) for h in range(2)]

                    def emit_dn_step():
                        if dn_steps:
                            fq, h = dn_steps.pop(0)
                            merge_step_dve(dnwin, dnw[fq, h],
                                           agd_in[fq, h])
                            if not dn_steps:
                                nc.gpsimd.collective_compute(
                                    "AllGather", mybir.AluOpType.bypass,
                                    replica_groups=rg,
                                    ins=[agd_in[:]], outs=[agd_out[:]],
                                )

                    for ci in range(len(AG_CHUNKS)):
                        up_gemm_chunk(wstrip, glu, gpsum, ci,
                                      after_pair=emit_dn_step)

                # ---- down GEMM ----

                with (
                    tc.tile_pool(name="dwstrip", bufs=2) as dwstrip,
                    tc.tile_pool(name="opool", bufs=3) as opool,
                    tc.tile_pool(name="dpsum", bufs=2, space="PSUM") as dpsum,
                ):
                    for dp in range(4):
                        wd = dwstrip.tile([P, FKS, 256], BF16)
                        nc.sync.dma_start(
                            wd[:],
                            agd_out[:, :, dp // 2, :, ds((dp % 2) * 256, 256)]
                            .rearrange("r q p o -> p (r q) o"))
                        for dtl in range(2):
                            dt = 2 * dp + dtl
                            ps_o0 = dpsum.tile([P, 512], F32, tag="ps_o0")
                            ps_o1 = dpsum.tile([P, 512], F32, tag="ps_o1")
                            for kt in range(FKS):
                                nc.tensor.matmul(ps_o0[:],
                                                 wd[:, kt, ds(dtl * P, P)],
                                                 st_slice(kt, ds(0, 512)),
                                                 start=(kt == 0),
                                                 stop=(kt == FKS - 1))
                                nc.tensor.matmul(ps_o1[:],
                                                 wd[:, kt, ds(dtl * P, P)],
                                                 st_slice(kt, ds(512, 512)),
                                                 start=(kt == 0),
                                                 stop=(kt == FKS - 1))
                            for th, ps_ox in ((0, ps_o0), (1, ps_o1)):
                                ot = opool.tile([P, 512], F32, tag="ot")
                                nc.vector.tensor_scalar_add(
                                    ot[:], ps_ox[:], bdn_sb[:, dt:dt + 1])
                                nc.scalar.dma_start(
                                    out[ds(dt * P, P), ds(th * 512, 512)], ot[:])

            mstack.close()

    nc.compile()
    return nc


def _get_nc():
    if "nc" not in _CACHED:
        _CACHED["nc"] = _build()
    return _CACHED["nc"]


def _prep(x, router_w, router_b, up_w, up_b, down_w, down_b):
    x = np.asarray(x, dtype=np.float32)
    router_w = np.asarray(router_w, dtype=np.float32)
    router_b = np.asarray(router_b, dtype=np.float32)
    up_w = np.asarray(up_w, dtype=np.float32)
    up_b = np.asarray(up_b, dtype=np.float32)
    down_w = np.asarray(down_w, dtype=np.float32)
    down_b = np.asarray(down_b, dtype=np.float32)

    bf = ml_dtypes.bfloat16

    # o-permutation: [L0, G0, L1, G1, ...] blocks of 128 rows
    perm = np.empty(DFF, dtype=np.int64)
    for t in range(F // P):
        perm[2 * t * P:(2 * t + 1) * P] = np.arange(t * P, (t + 1) * P)
        perm[(2 * t + 1) * P:(2 * t + 2) * P] = np.arange(F + t * P, F + (t + 1) * P)

    xf = x.reshape(T, D)
    up_bf = up_w.astype(bf)[:, perm, :]          # [E, DFF(perm), D]
    dn_bf = down_w.astype(bf)                    # [E, D, F]

    rwb = np.zeros((RWB_ROWS, E), dtype=np.float32)
    rwb[:D] = router_w.T
    rwb[D] = router_b
    upb_p = np.zeros((P, DFF), dtype=np.float32)
    upb_p[:E] = up_b[:, perm]
    dnb_c = np.zeros((P, D), dtype=np.float32)
    dnb_c[:E] = down_b

    in_maps = []
    for c in range(NCORES):
        xt_c = np.ascontiguousarray(xf[c * TPC:(c + 1) * TPC].T)
        # upw chunk-contiguous: [oc, p(d), e, o]
        upw_c = np.ascontiguousarray(
            up_bf[:, :, c * P:(c + 1) * P]
            .reshape(E, DFF // 512, 512, P).transpose(1, 3, 0, 2))
        # dnw chunk-contiguous: [fq, h, p(f), e, o(d')]
        dnw_c = np.ascontiguousarray(
            dn_bf[:, :, c * (F // NCORES):(c + 1) * (F // NCORES)]
            .transpose(0, 2, 1)                      # [E, 512(f), D]
            .reshape(E, 4, P, 2, 512).transpose(1, 3, 2, 0, 4))
        bsel_c = np.zeros((P, B), dtype=np.float32)
        bsel_c[:, c // 2] = 1.0
        in_maps.append({
            "xt": xt_c, "rwb": rwb, "upw": upw_c, "dnw": dnw_c,
            "upb": upb_p, "dnb": dnb_c, "bsel": bsel_c,
        })
    return in_maps


def kernel(x, router_w, router_b, up_w, up_b, down_w, down_b):
    in_maps = _prep(x, router_w, router_b, up_w, up_b, down_w, down_b)
    nc = _get_nc()
    res = run_bass_kernel_spmd(nc, in_maps, core_ids=list(range(NCORES)))

    outp = np.empty((T, D), dtype=np.float32)
    for c in range(NCORES):
        outp[c * TPC:(c + 1) * TPC] = res.results[c]["out"].T
    return outp.reshape(B, S, D)
